# revision 2
# baseline (speedup 1.0000x reference)
"""Trainium2 Bass kernel for nn_BertL2PredictionHead: out = -||x - emb||_2 + bias.

Computes out[b,s,v] = bias[v] - sqrt(max(||x_bs||^2 + ||emb_v||^2 - 2 x_bs.emb_v, 0))
for x (16,128,128) f32, emb (20001,128) f32, bias (1,1,20001) f32.

Sharding: vocab dimension split across 8 NeuronCores (tensor parallel over V),
x replicated. Per core: a [2048 x 2501] slice of the distance matrix via
  psum  = (-2 x^T)^T @ embT        (f32r / TF32 matmul, full PE rate)
        + ones^T @ [esq_hi; esq_lo] (K=2 bf16 rank-1, adds ||emb_v||^2)
  sbuf  = Sqrt(psum + ||x_m||^2)    (ACT, per-partition bias)
  out   = -sbuf                     (DVE tensor_scalar, 2x mode)
then one contiguous 1.28 MB DMA per 128-row tile. Host splits/pads inputs,
gathers the 8 output slices, and applies bias (zero in practice) at the end.
"""
import sys

sys.path.insert(0, "/opt/trn_rl_repo")

import numpy as np
import ml_dtypes
from contextlib import ExitStack

import concourse.bass as bass  # noqa: F401  (bass types used via tile/bacc)
import concourse.tile as tile
from concourse import bacc, mybir
from concourse.bass_utils import run_bass_kernel_spmd

F32 = mybir.dt.float32
F32R = mybir.dt.float32r
BF16 = mybir.dt.bfloat16

NCORES = 8
B, S, H, V = 16, 128, 128, 20001
BS = B * S                      # 2048 rows
MT = BS // 128                  # 16 m-tiles of 128 rows
VS = 2502                       # vocab slice per core (even: fp32r needs even free dims)
VPAD = VS * NCORES              # 20008
CHUNK = 512
CHUNKS = [(c, min(CHUNK, VS - c)) for c in range(0, VS, CHUNK)]


def _tf32(a: np.ndarray) -> np.ndarray:
    """Round fp32 to TF32 (10-bit mantissa, round-to-nearest-even)."""
    u = a.view(np.uint32).astype(np.uint64)
    lsb = (u >> 13) & 1
    u2 = (u + 0x0FFF + lsb) & 0xFFFFFFFF
    return (u2 & ~np.uint64(0x1FFF)).astype(np.uint32).view(np.float32)


_PROG = None  # (nc,) compiled once per process


def _build():
    global _PROG
    if _PROG is not None:
        return _PROG

    nc = bacc.Bacc("TRN2", target_bir_lowering=False, debug=False)

    xT2_d = nc.dram_tensor("xT2", [H, BS], F32R, kind="ExternalInput").ap()
    embT_d = nc.dram_tensor("embT", [H, VS], F32R, kind="ExternalInput").ap()
    esq2_d = nc.dram_tensor("esq2", [2, VS], BF16, kind="ExternalInput").ap()
    xsqc_d = nc.dram_tensor("xsqc", [128, MT], F32, kind="ExternalInput").ap()
    out_d = nc.dram_tensor("out", [BS, VS], F32, kind="ExternalOutput").ap()

    with tile.TileContext(nc) as tc, ExitStack() as ctx:
        const = ctx.enter_context(tc.tile_pool(name="const", bufs=1))
        opool = ctx.enter_context(tc.tile_pool(name="opool", bufs=2))
        psum = ctx.enter_context(tc.tile_pool(name="psum", bufs=8, space="PSUM"))

        # Inputs staged in SBUF for the whole kernel. Separate tiles per
        # chunk so each matmul only waits on the slice it reads.
        xt_tiles = []
        for t in range(MT):
            xt = const.tile([H, 128], F32R, tag=f"xt{t}")
            nc.sync.dma_start(out=xt[:], in_=xT2_d[:, t * 128:(t + 1) * 128])
            xt_tiles.append(xt)
        emb_tiles = []
        for i, (c0, w) in enumerate(CHUNKS):
            et = const.tile([H, w], F32R, tag=f"emb{i}")
            nc.sync.dma_start(out=et[:], in_=embT_d[:, c0:c0 + w])
            emb_tiles.append(et)
        esq2_s = const.tile([2, VS], BF16)
        nc.sync.dma_start(out=esq2_s[:], in_=esq2_d[:])
        xsqc_s = const.tile([128, MT], F32)
        nc.sync.dma_start(out=xsqc_s[:], in_=xsqc_d[:])
        ones2 = const.tile([2, 128], BF16)
        nc.vector.memset(ones2[:], 1.0)

        for t in range(MT):
            o_t = opool.tile([128, VS], F32, tag="o")
            for i, (c0, w) in enumerate(CHUNKS):
                ps = psum.tile([128, CHUNK], F32, tag="ps")
                nc.tensor.matmul(ps[:, :w], xt_tiles[t][:], emb_tiles[i][:],
                                 start=True, stop=False)
                nc.tensor.matmul(ps[:, :w], ones2[:], esq2_s[:, c0:c0 + w],
                                 start=False, stop=True)
                nc.scalar.activation(o_t[:, c0:c0 + w], ps[:, :w],
                                     mybir.ActivationFunctionType.Sqrt,
                                     bias=xsqc_s[:, t:t + 1], scale=1.0)
                nc.vector.tensor_scalar_mul(o_t[:, c0:c0 + w], o_t[:, c0:c0 + w],
                                            -1.0)
            nc.sync.dma_start(out=out_d[t * 128:(t + 1) * 128, :], in_=o_t[:])

    nc.compile()
    _PROG = (nc,)
    return _PROG


def _prep_in_maps(x: np.ndarray, emb: np.ndarray):
    X = np.asarray(x, dtype=np.float32).reshape(BS, H)
    xT2 = _tf32(np.ascontiguousarray(X.T) * np.float32(-2.0))
    xsq = (X.astype(np.float64) ** 2).sum(axis=1).astype(np.float32)
    xsqc = np.ascontiguousarray(xsq.reshape(MT, 128).T)   # [128, MT]

    embp = np.zeros((VPAD, H), dtype=np.float32)
    embp[:V] = np.asarray(emb, dtype=np.float32)
    embT = _tf32(np.ascontiguousarray(embp.T))            # [H, VPAD]
    esq = (embp.astype(np.float64) ** 2).sum(axis=1).astype(np.float32)
    esq_hi = esq.astype(ml_dtypes.bfloat16)
    esq_lo = (esq - esq_hi.astype(np.float32)).astype(ml_dtypes.bfloat16)
    esq2 = np.stack([esq_hi, esq_lo], axis=0)             # [2, VPAD] bf16

    maps = []
    for c in range(NCORES):
        lo = c * VS
        maps.append({
            "xT2": xT2,
            "embT": np.ascontiguousarray(embT[:, lo:lo + VS]),
            "esq2": np.ascontiguousarray(esq2[:, lo:lo + VS]),
            "xsqc": xsqc,
        })
    return maps


def _run_cores(in_maps, trace: bool = False):
    (nc,) = _build()
    return run_bass_kernel_spmd(nc, in_maps, list(range(NCORES)), trace=trace)


def kernel(x: np.ndarray, emb: np.ndarray, bias: np.ndarray) -> np.ndarray:
    in_maps = _prep_in_maps(x, emb)
    res = _run_cores(in_maps)

    out = np.empty((BS, V), dtype=np.float32)
    for c in range(NCORES):
        lo = c * VS
        hi = min(lo + VS, V)
        out[:, lo:hi] = res.results[c]["out"][:, :hi - lo]
    out = out.reshape(B, S, V)

    bias_np = np.asarray(bias, dtype=np.float32)
    if np.any(bias_np):
        out = out + bias_np.reshape(1, 1, V)
    return out


# revision 4
# speedup vs baseline: 1.0324x; 1.0324x over previous
"""Trainium2 Bass kernel for nn_BertL2PredictionHead: out = -||x - emb||_2 + bias.

out[b,s,v] = bias[v] - sqrt(max(||x_bs||^2 + ||emb_v||^2 - 2 x_bs.emb_v, 0))
for x (16,128,128) f32, emb (20001,128) f32, bias (1,1,20001) f32.

Sharding: vocab dimension split across 8 NeuronCores (tensor parallel over V),
x replicated. Per core, for each 128-row tile of the 2048x2502 slice:
  psum = (-2 x^T)^T @ embT          f32r (TF32) matmul, 1 cyc/col, same
                                    stationary weights for all 5 chunks
  o    = psum + esq_bcast           DVE tensor_tensor, adds ||emb_v||^2 (fp32)
  o    = Sqrt(o + ||x_m||^2)        one wide ACT per m-tile, per-partition bias
  DMA o -> out slice                one contiguous 1.28 MB store per m-tile
The final negation and the bias add are fused into the host-side gather copy
(np.negative / np.subtract with out=), which costs nothing beyond the copy.
"""
import sys

sys.path.insert(0, "/opt/trn_rl_repo")

import numpy as np
from contextlib import ExitStack

import concourse.bass as bass  # noqa: F401
import concourse.tile as tile
from concourse import bacc, mybir
from concourse.bass_utils import run_bass_kernel_spmd

F32 = mybir.dt.float32
F32R = mybir.dt.float32r

NCORES = 8
B, S, H, V = 16, 128, 128, 20001
BS = B * S                      # 2048 rows
MT = BS // 128                  # 16 m-tiles of 128 rows
VS = 2502                       # vocab slice per core (even: fp32r needs even free dims)
VPAD = VS * NCORES              # 20016
# psum tiles per m-tile: two [128,1024] (2 banks each) + one [128,454]
PW = 1024
TAIL = VS - 2 * PW              # 454


def _tf32(a: np.ndarray) -> np.ndarray:
    """Round fp32 to TF32 (10-bit mantissa, round-to-nearest-even)."""
    u = a.view(np.uint32).astype(np.uint64)
    lsb = (u >> 13) & 1
    u2 = (u + 0x0FFF + lsb) & 0xFFFFFFFF
    return (u2 & ~np.uint64(0x1FFF)).astype(np.uint32).view(np.float32)


_PROG = None  # (nc,) compiled once per process


def _build():
    global _PROG
    if _PROG is not None:
        return _PROG

    nc = bacc.Bacc("TRN2", target_bir_lowering=False, debug=False)

    xT2_d = nc.dram_tensor("xT2", [H, BS], F32R, kind="ExternalInput").ap()
    embT_d = nc.dram_tensor("embT", [H, VS], F32R, kind="ExternalInput").ap()
    esq_d = nc.dram_tensor("esq", [1, VS], F32, kind="ExternalInput").ap()
    xsqc_d = nc.dram_tensor("xsqc", [128, MT], F32, kind="ExternalInput").ap()
    out_d = nc.dram_tensor("out", [BS, VS], F32, kind="ExternalOutput").ap()

    with tile.TileContext(nc) as tc, ExitStack() as ctx:
        const = ctx.enter_context(tc.tile_pool(name="const", bufs=1))
        opool = ctx.enter_context(tc.tile_pool(name="opool", bufs=2))
        psum = ctx.enter_context(tc.tile_pool(name="psum", bufs=1, space="PSUM"))

        # Staged inputs. Two DMAs per big tensor: a small head so the first
        # matmuls can start early (subtile deps), then the rest.
        xt_s = const.tile([H, BS], F32R)
        nc.sync.dma_start(out=xt_s[:, 0:512], in_=xT2_d[:, 0:512])
        emb_s = const.tile([H, VS], F32R)
        nc.sync.dma_start(out=emb_s[:, 0:PW], in_=embT_d[:, 0:PW])
        xsqc_s = const.tile([128, MT], F32)
        nc.sync.dma_start(out=xsqc_s[:], in_=xsqc_d[:])
        nc.sync.dma_start(out=xt_s[:, 512:BS], in_=xT2_d[:, 512:BS])
        nc.sync.dma_start(out=emb_s[:, PW:VS], in_=embT_d[:, PW:VS])

        # esq broadcast to all 128 partitions (one-time, replicating DMA)
        esqb = const.tile([128, VS], F32)
        nc.sync.dma_start(out=esqb[:], in_=esq_d[:].broadcast_to([128, VS]))

        for t in range(MT):
            o_t = opool.tile([128, VS], F32, tag="o", name=f"o{t}")
            xt = xt_s[:, t * 128:(t + 1) * 128]
            for g in range(2):
                pw = psum.tile([128, PW], F32, tag="pw", bufs=3, name=f"pw{t}_{g}")
                for h in range(2):
                    c0 = g * PW + h * 512
                    nc.tensor.matmul(pw[:, h * 512:(h + 1) * 512], xt,
                                     emb_s[:, c0:c0 + 512], start=True, stop=True)
                nc.vector.tensor_add(o_t[:, g * PW:(g + 1) * PW], pw[:],
                                     esqb[:, g * PW:(g + 1) * PW])
            pt = psum.tile([128, TAIL], F32, tag="pt", bufs=2, name=f"pt{t}")
            nc.tensor.matmul(pt[:], xt, emb_s[:, 2 * PW:VS], start=True, stop=True)
            nc.vector.tensor_add(o_t[:, 2 * PW:VS], pt[:], esqb[:, 2 * PW:VS])

            nc.scalar.activation(o_t[:], o_t[:],
                                 mybir.ActivationFunctionType.Sqrt,
                                 bias=xsqc_s[:, t:t + 1], scale=1.0)
            nc.sync.dma_start(out=out_d[t * 128:(t + 1) * 128, :], in_=o_t[:])

    nc.compile()
    _PROG = (nc,)
    return _PROG


def _prep_in_maps(x: np.ndarray, emb: np.ndarray):
    X = np.asarray(x, dtype=np.float32).reshape(BS, H)
    xT2 = _tf32(np.ascontiguousarray(X.T) * np.float32(-2.0))
    xsq = (X.astype(np.float64) ** 2).sum(axis=1).astype(np.float32)
    xsqc = np.ascontiguousarray(xsq.reshape(MT, 128).T)   # [128, MT]

    embp = np.zeros((VPAD, H), dtype=np.float32)
    embp[:V] = np.asarray(emb, dtype=np.float32)
    embT = _tf32(np.ascontiguousarray(embp.T))            # [H, VPAD]
    esq = (embp.astype(np.float64) ** 2).sum(axis=1).astype(np.float32)

    maps = []
    for c in range(NCORES):
        lo = c * VS
        maps.append({
            "xT2": xT2,
            "embT": np.ascontiguousarray(embT[:, lo:lo + VS]),
            "esq": np.ascontiguousarray(esq[lo:lo + VS].reshape(1, VS)),
            "xsqc": xsqc,
        })
    return maps


def _run_cores(in_maps, trace: bool = False):
    (nc,) = _build()
    return run_bass_kernel_spmd(nc, in_maps, list(range(NCORES)), trace=trace)


def kernel(x: np.ndarray, emb: np.ndarray, bias: np.ndarray) -> np.ndarray:
    in_maps = _prep_in_maps(x, emb)
    res = _run_cores(in_maps)

    bias_np = np.asarray(bias, dtype=np.float32).reshape(-1)
    have_bias = bool(np.any(bias_np))

    # Gather + fused negate (+ bias): out = bias - dist
    out = np.empty((BS, V), dtype=np.float32)
    for c in range(NCORES):
        lo = c * VS
        hi = min(lo + VS, V)
        dist = res.results[c]["out"][:, :hi - lo]
        if have_bias:
            np.subtract(bias_np[lo:hi][None, :], dist, out=out[:, lo:hi])
        else:
            np.negative(dist, out=out[:, lo:hi])
    return out.reshape(B, S, V)


# revision 5
# speedup vs baseline: 1.6195x; 1.5687x over previous
"""Trainium2 Bass kernel for nn_BertL2PredictionHead: out = -||x - emb||_2 + bias.

out[b,s,v] = bias[v] - sqrt(max(||x_bs||^2 + ||emb_v||^2 - 2 x_bs.emb_v, 0))
for x (16,128,128) f32, emb (20001,128) f32, bias (1,1,20001) f32.

Sharding: vocab dimension split across 8 NeuronCores (tensor parallel over V),
x replicated. Per core, for each 128-row tile of the 2048x2502 slice:
  psum = (-2 x^T)^T @ embT          f32r (TF32) matmul, 1 cyc/col, same
                                    stationary weights for all 5 chunks
  o    = psum + esq_bcast           DVE tensor_tensor, adds ||emb_v||^2 (fp32)
  o    = Sqrt(o + ||x_m||^2)        one wide ACT per m-tile, per-partition bias
  DMA o -> out slice                one contiguous 1.28 MB store per m-tile
The final negation and the bias add are fused into the host-side gather copy
(np.negative / np.subtract with out=), which costs nothing beyond the copy.
"""
import sys

sys.path.insert(0, "/opt/trn_rl_repo")

import numpy as np
from contextlib import ExitStack

import concourse.bass as bass  # noqa: F401
import concourse.tile as tile
from concourse import bacc, mybir
from concourse.bass_utils import run_bass_kernel_spmd

F32 = mybir.dt.float32
F32R = mybir.dt.float32r

NCORES = 8
B, S, H, V = 16, 128, 128, 20001
BS = B * S                      # 2048 rows
MT = BS // 128                  # 16 m-tiles of 128 rows
VS = 2502                       # vocab slice per core (even: fp32r needs even free dims)
VPAD = VS * NCORES              # 20016
# psum tiles per m-tile: two [128,1024] (2 banks each) + one [128,454]
PW = 1024
TAIL = VS - 2 * PW              # 454


def _tf32(a: np.ndarray) -> np.ndarray:
    """Round fp32 to TF32 (10-bit mantissa, round-to-nearest-even)."""
    u = a.view(np.uint32).astype(np.uint64)
    lsb = (u >> 13) & 1
    u2 = (u + 0x0FFF + lsb) & 0xFFFFFFFF
    return (u2 & ~np.uint64(0x1FFF)).astype(np.uint32).view(np.float32)


_PROG = None  # (nc,) compiled once per process


def _build():
    global _PROG
    if _PROG is not None:
        return _PROG

    nc = bacc.Bacc("TRN2", target_bir_lowering=False, debug=False)

    xT2_d = nc.dram_tensor("xT2", [H, BS], F32R, kind="ExternalInput").ap()
    embT_d = nc.dram_tensor("embT", [H, VS], F32R, kind="ExternalInput").ap()
    esq_d = nc.dram_tensor("esq", [1, VS], F32, kind="ExternalInput").ap()
    xsqc_d = nc.dram_tensor("xsqc", [128, MT], F32, kind="ExternalInput").ap()
    out_d = nc.dram_tensor("out", [BS, VS], F32, kind="ExternalOutput").ap()

    with tile.TileContext(nc) as tc, ExitStack() as ctx:
        const = ctx.enter_context(tc.tile_pool(name="const", bufs=1))
        opool = ctx.enter_context(tc.tile_pool(name="opool", bufs=4))
        psum = ctx.enter_context(tc.tile_pool(name="psum", bufs=1, space="PSUM"))

        # Staged inputs. Two DMAs per big tensor: a small head so the first
        # matmuls can start early (subtile deps), then the rest.
        xt_s = const.tile([H, BS], F32R)
        nc.sync.dma_start(out=xt_s[:, 0:512], in_=xT2_d[:, 0:512])
        emb_s = const.tile([H, VS], F32R)
        nc.sync.dma_start(out=emb_s[:, 0:PW], in_=embT_d[:, 0:PW])
        xsqc_s = const.tile([128, MT], F32)
        nc.sync.dma_start(out=xsqc_s[:], in_=xsqc_d[:])
        nc.sync.dma_start(out=xt_s[:, 512:BS], in_=xT2_d[:, 512:BS])
        nc.sync.dma_start(out=emb_s[:, PW:VS], in_=embT_d[:, PW:VS])

        # esq broadcast to all 128 partitions (one-time, replicating DMA)
        esqb = const.tile([128, VS], F32)
        nc.sync.dma_start(out=esqb[:], in_=esq_d[:].broadcast_to([128, VS]))

        for t in range(MT):
            o_t = opool.tile([128, VS], F32, tag="o", name=f"o{t}")
            xt = xt_s[:, t * 128:(t + 1) * 128]
            for g in range(2):
                pw = psum.tile([128, PW], F32, tag="pw", bufs=3, name=f"pw{t}_{g}")
                for h in range(2):
                    c0 = g * PW + h * 512
                    nc.tensor.matmul(pw[:, h * 512:(h + 1) * 512], xt,
                                     emb_s[:, c0:c0 + 512], start=True, stop=True)
                nc.vector.tensor_add(o_t[:, g * PW:(g + 1) * PW], pw[:],
                                     esqb[:, g * PW:(g + 1) * PW])
            pt = psum.tile([128, TAIL], F32, tag="pt", bufs=2, name=f"pt{t}")
            nc.tensor.matmul(pt[:], xt, emb_s[:, 2 * PW:VS], start=True, stop=True)
            nc.vector.tensor_add(o_t[:, 2 * PW:VS], pt[:], esqb[:, 2 * PW:VS])

            for (c0, c1) in ((0, PW), (PW, 2 * PW), (2 * PW, VS)):
                nc.scalar.activation(o_t[:, c0:c1], o_t[:, c0:c1],
                                     mybir.ActivationFunctionType.Sqrt,
                                     bias=xsqc_s[:, t:t + 1], scale=1.0)
            nc.sync.dma_start(out=out_d[t * 128:(t + 1) * 128, :], in_=o_t[:])

    nc.compile()
    _PROG = (nc,)
    return _PROG


def _prep_in_maps(x: np.ndarray, emb: np.ndarray):
    X = np.asarray(x, dtype=np.float32).reshape(BS, H)
    xT2 = _tf32(np.ascontiguousarray(X.T) * np.float32(-2.0))
    xsq = (X.astype(np.float64) ** 2).sum(axis=1).astype(np.float32)
    xsqc = np.ascontiguousarray(xsq.reshape(MT, 128).T)   # [128, MT]

    embp = np.zeros((VPAD, H), dtype=np.float32)
    embp[:V] = np.asarray(emb, dtype=np.float32)
    embT = _tf32(np.ascontiguousarray(embp.T))            # [H, VPAD]
    esq = (embp.astype(np.float64) ** 2).sum(axis=1).astype(np.float32)

    maps = []
    for c in range(NCORES):
        lo = c * VS
        maps.append({
            "xT2": xT2,
            "embT": np.ascontiguousarray(embT[:, lo:lo + VS]),
            "esq": np.ascontiguousarray(esq[lo:lo + VS].reshape(1, VS)),
            "xsqc": xsqc,
        })
    return maps


def _run_cores(in_maps, trace: bool = False):
    (nc,) = _build()
    return run_bass_kernel_spmd(nc, in_maps, list(range(NCORES)), trace=trace)


def kernel(x: np.ndarray, emb: np.ndarray, bias: np.ndarray) -> np.ndarray:
    in_maps = _prep_in_maps(x, emb)
    res = _run_cores(in_maps)

    bias_np = np.asarray(bias, dtype=np.float32).reshape(-1)
    have_bias = bool(np.any(bias_np))

    # Gather + fused negate (+ bias): out = bias - dist
    out = np.empty((BS, V), dtype=np.float32)
    for c in range(NCORES):
        lo = c * VS
        hi = min(lo + VS, V)
        dist = res.results[c]["out"][:, :hi - lo]
        if have_bias:
            np.subtract(bias_np[lo:hi][None, :], dist, out=out[:, lo:hi])
        else:
            np.negative(dist, out=out[:, lo:hi])
    return out.reshape(B, S, V)
